# revision 1
# baseline (speedup 1.0000x reference)
"""Trainium2 Bass kernel for nn_CausalAttention (N=4096, 8 heads, DH=32).

Strategy: head-parallel across 8 NeuronCores (1 head per core).
Per core:
  - QKV projections from channels-major inputs [256, 4096] (natural layout
    is already the transposed layout the TensorEngine wants), fp32r.
  - Scores computed transposed: S^T[k, q] = K @ Q^T, in 512-query blocks,
    3 k-tiles (128 keys each) per PSUM group via row-packed K=32 matmuls.
    Diagonal sub-tiles skip their fully-masked column prefix.
  - Max-free softmax: P^T = exp(S / sqrt(32)) with strict-causal 0/1 mask
    applied post-exp (scores are O(1), so exp never overflows; reference's
    -10000 masking underflows to exactly 0 in f32, matching the 0-mask).
  - Softmax denominator folded into the PV matmul via a ones column
    appended to V (lhsT [128, 33]); P^T and V in bf16 (f32 accumulate).
  - Normalization without transposes: colsum -> [8, 64] (reshape DMA) ->
    reciprocal -> [1, 512] (reshape DMA) -> K=1 matmul replicate to
    [32, 512] PSUM -> one tensor_mul. Output stays in O^T layout
    [32, 4096] per core; host reshapes to [1, 256, 64, 64].
"""

import math

import numpy as np
import ml_dtypes

import concourse.bass as bass
import concourse.mybir as mybir
from concourse import bacc
from concourse.tile import TileContext
from concourse.bass_utils import run_bass_kernel_spmd

# Problem constants (hardcoded per harness contract).
B, CQ, CK, CH, NH, H, W = 1, 256, 256, 256, 8, 64, 64
DH = CH // NH            # 32
N = H * W                # 4096
QB = 512                 # queries per block
NQB = N // QB            # 8
KT = 128                 # keys per k-tile
NKT = N // KT            # 32
GS = 2                   # k-tiles per S-group (2 PSUM banks per group)
NG = (NKT + GS - 1) // GS  # 11 column-groups in packed kT layout
SCALE = 1.0 / math.sqrt(DH)

F32 = mybir.dt.float32
F32R = mybir.dt.float32r
BF16 = mybir.dt.bfloat16

_CACHED_NC = None


def _build():
    nc = bacc.Bacc("TRN2", target_bir_lowering=False, debug=False, num_devices=1)

    qin_d = nc.dram_tensor("qin", [CQ, N], F32, kind="ExternalInput")
    kin_d = nc.dram_tensor("kin", [CK, N], F32, kind="ExternalInput")
    wq_d = nc.dram_tensor("wqt", [CQ, 128], F32, kind="ExternalInput")
    wk_d = nc.dram_tensor("wkt", [CK, 128], F32, kind="ExternalInput")
    wv_d = nc.dram_tensor("wvt", [CK, DH], F32, kind="ExternalInput")
    bq_d = nc.dram_tensor("bqr", [128, 1], F32, kind="ExternalInput")
    bk_d = nc.dram_tensor("bkr", [128, 1], F32, kind="ExternalInput")
    bv_d = nc.dram_tensor("bvr", [128, DH], F32, kind="ExternalInput")
    out_d = nc.dram_tensor("out", [DH, N], F32, kind="ExternalOutput")

    # Strict-causal mask window: tm[kk, j] = 1.0 iff kk < j - 384; the
    # [*, 384:512] slice gives mask[kk, qq] = (kk < qq) for the 128-wide
    # diagonal window.
    tm_np = (np.arange(128)[:, None] < (np.arange(512)[None, :] - 384)).astype(
        ml_dtypes.bfloat16
    )
    tm_d = nc.inline_tensor(tm_np, name="tmask")
    ones_d = nc.inline_tensor(np.ones((1, DH), dtype=np.float32), name="onesd")
    idb_d = nc.inline_tensor(np.eye(DH, dtype=ml_dtypes.bfloat16), name="idbd")

    with TileContext(nc) as tc:
        with (
            tc.tile_pool(name="constp", bufs=1) as constp,
            tc.tile_pool(name="bigp", bufs=1) as bigp,
            tc.tile_pool(name="workp", bufs=4) as workp,
            tc.tile_pool(name="spool", bufs=3, space="PSUM") as spool,
            tc.tile_pool(name="mpool", bufs=2, space="PSUM") as mpool,
        ):
            # ---- DMAs: first input slices + first weights get priority ----
            kin_sb = bigp.tile([128, 2, N], F32R, name="kin_sb")
            qin_sb = bigp.tile([128, 2, N], F32R, name="qin_sb")
            kin_ap = kin_d.ap().rearrange("(c p) n -> p c n", p=128).bitcast(F32R)
            qin_ap = qin_d.ap().rearrange("(c p) n -> p c n", p=128).bitcast(F32R)
            slA, slB = slice(0, 512), slice(512, 1024)
            for ch in range(2):
                nc.sync.dma_start(kin_sb[:, ch, slA], kin_ap[:, ch, slA])
            wk_sb = constp.tile([128, 2, 128], F32R, name="wk_sb")
            nc.scalar.dma_start(
                wk_sb[:], wk_d.ap().rearrange("(c p) m -> p c m", p=128).bitcast(F32R)
            )
            wq_sb = constp.tile([128, 2, 128], F32R, name="wq_sb")
            nc.scalar.dma_start(
                wq_sb[:], wq_d.ap().rearrange("(c p) m -> p c m", p=128).bitcast(F32R)
            )
            for ch in range(2):
                nc.scalar.dma_start(qin_sb[:, ch, slA], qin_ap[:, ch, slA])
            for ch in range(2):
                nc.sync.dma_start(kin_sb[:, ch, slB], kin_ap[:, ch, slB])
            for ch in range(2):
                nc.scalar.dma_start(qin_sb[:, ch, slB], qin_ap[:, ch, slB])
            wv_sb = constp.tile([128, 2, DH], F32R, name="wv_sb")
            nc.scalar.dma_start(
                wv_sb[:], wv_d.ap().rearrange("(c p) m -> p c m", p=128).bitcast(F32R)
            )
            bq_sb = constp.tile([128, 1], F32, name="bq_sb")
            nc.scalar.dma_start(bq_sb[:], bq_d.ap())
            bk_sb = constp.tile([128, 1], F32, name="bk_sb")
            nc.scalar.dma_start(bk_sb[:], bk_d.ap())
            bv_sb = constp.tile([128, DH], F32, name="bv_sb")
            nc.scalar.dma_start(bv_sb[:], bv_d.ap())
            tm_sb = constp.tile([128, 512], BF16, name="tm_sb")
            nc.scalar.dma_start(tm_sb[:], tm_d.ap())
            ones_sb = constp.tile([1, DH], F32R, name="ones_sb")
            nc.scalar.dma_start(ones_sb[:], ones_d.ap().bitcast(F32R))
            idb_sb = constp.tile([DH, DH], BF16, name="idb_sb")
            nc.scalar.dma_start(idb_sb[:], idb_d.ap())
            for h in range(1, 4):
                sl = slice(1024 * h, 1024 * (h + 1))
                for ch in range(2):
                    nc.sync.dma_start(kin_sb[:, ch, sl], kin_ap[:, ch, sl])
                for ch in range(2):
                    nc.scalar.dma_start(qin_sb[:, ch, sl], qin_ap[:, ch, sl])

            # ---- projections (interleaved per 512-slice for fast start) ----
            # kT3[32u+d, 128g+kk] = k^T[d, 128*(3g+u)+kk]  (3-way row packing)
            kT3 = bigp.tile([32 * GS, NG * 128], F32R, name="kT3")
            # qT[32u+d, q] = q^T[d, q] for u=0..3 (4x replicated on partitions)
            qT = bigp.tile([128, N], F32R, name="qT")
            # v_all[kk, t, :DH] = v[128t+kk, :]; col DH is the ones column
            # (inner dim padded to 48 so transpose-DMA dests are 32B-aligned)
            v_all = bigp.tile([128, NKT, 48], BF16, name="v_all")
            nc.vector.memset(v_all[:, :, DH : DH + 1], 1.0)

            stage_q = []  # deferred tail stages, advanced one per group

            def emit_proj(s):
                ksl = slice(512 * s, 512 * (s + 1))
                pj = spool.tile([128, 512], F32, name="pj", tag="s")
                for ch in range(2):
                    nc.tensor.matmul(
                        pj[:],
                        wk_sb[:, ch, :],
                        kin_sb[:, ch, ksl],
                        start=(ch == 0),
                        stop=(ch == 1),
                    )
                for ci in range(4):
                    j = 4 * s + ci
                    u, g = j % GS, j // GS
                    nc.vector.tensor_scalar_add(
                        kT3[32 * u : 32 * u + 32, 128 * g : 128 * g + 128],
                        pj[32 * u : 32 * u + 32, 128 * ci : 128 * ci + 128],
                        bk_sb[32 * u : 32 * u + 32, :],
                    )
                pj = spool.tile([128, 512], F32, name="pj", tag="s")
                for ch in range(2):
                    nc.tensor.matmul(
                        pj[:],
                        wq_sb[:, ch, :],
                        qin_sb[:, ch, ksl],
                        start=(ch == 0),
                        stop=(ch == 1),
                    )
                nc.vector.tensor_scalar_add(qT[:, ksl], pj[:], bq_sb[:])
                for t in range(4 * s, 4 * s + 4):
                    nsl = slice(128 * t, 128 * (t + 1))
                    pj = spool.tile([128, DH], F32, name="pj", tag="s")
                    for ch in range(2):
                        nc.tensor.matmul(
                            pj[:],
                            kin_sb[:, ch, nsl],
                            wv_sb[:, ch, :],
                            start=(ch == 0),
                            stop=(ch == 1),
                        )
                    nc.vector.tensor_add(v_all[:, t, 0:DH], pj[:], bv_sb[:])

            for s in range(8):
                emit_proj(s)

            # ---- block tails, staged into the following block's stream so
            # the in-order PE/DVE never stall on the reciprocal DMA chain ----
            def tail_a(st):
                o_ps = st["o_ps"]
                o_sb = workp.tile([DH, 512], F32, name="o_sb")
                nc.vector.tensor_copy(o_sb[:], o_ps[0:DH, :])
                cs_sb = workp.tile([1, 512], F32, name="cs_sb")
                # +1e-30 keeps q=0 (fully masked row) at 0 instead of NaN
                nc.vector.tensor_scalar_add(cs_sb[:], o_ps[DH : DH + 1, :], 1e-30)
                cs8 = workp.tile([8, 64], F32, name="cs8")
                nc.sync.dma_start(cs8[:], cs_sb[:])
                st.update(o_sb=o_sb, cs8=cs8)

            def tail_b(st):
                cs8r = workp.tile([8, 64], F32, name="cs8r")
                nc.vector.reciprocal(cs8r[:], st["cs8"][:])
                csr = workp.tile([1, 512], F32R, name="csr")
                nc.sync.dma_start(csr[:], cs8r[:].bitcast(F32R))
                st.update(csr=csr)

            def tail_c(st):
                qb = st["qb"]
                rep_ps = mpool.tile([DH, 512], F32, name="rep_ps", tag="m")
                nc.tensor.matmul(
                    rep_ps[:], ones_sb[:], st["csr"][:], start=True, stop=True
                )
                out_sb = workp.tile([DH, 512], F32, name="out_sb")
                nc.vector.tensor_mul(out_sb[:], st["o_sb"][:], rep_ps[:])
                nc.sync.dma_start(
                    out_d.ap()[:, 512 * qb : 512 * (qb + 1)], out_sb[:]
                )

            for qb in range(NQB):
                o_ps = mpool.tile([DH + 1, 512], F32, name="o_ps", tag="m")
                nkt_q = 4 * (qb + 1)          # causal: k-tiles 0..nkt_q-1
                ngr = (nkt_q + GS - 1) // GS
                pends = []                     # (g, nsub, p_sb) awaiting PV
                first = True

                def flush_pv(pend, first, last, o_ps=o_ps):
                    g, nsub, p_sb = pend
                    f = first
                    for u in range(nsub):
                        j = GS * g + u
                        nc.tensor.matmul(
                            o_ps[:],
                            v_all[:, j, 0 : DH + 1],
                            p_sb[:, 512 * u : 512 * (u + 1)],
                            start=f,
                            stop=(last and u == nsub - 1),
                            skip_group_check=True,
                        )
                        f = False
                    return False

                for g in range(ngr):
                    nsub = min(GS, nkt_q - GS * g)
                    s_ps = spool.tile([128, GS * 512], F32, name="s_ps", tag="s")
                    for u in range(nsub):
                        j = GS * g + u
                        o = max(0, 128 * j - 512 * qb)
                        nc.tensor.matmul(
                            s_ps[:, 512 * u + o : 512 * (u + 1)],
                            kT3[32 * u : 32 * u + 32, 128 * g : 128 * g + 128],
                            qT[32 * u : 32 * u + 32, 512 * qb + o : 512 * (qb + 1)],
                            start=True,
                            stop=True,
                        )
                    p_sb = workp.tile([128, GS * 512], BF16, name="p_sb", bufs=12)
                    nc.scalar.activation(
                        p_sb[:, 0 : 512 * nsub],
                        s_ps[:, 0 : 512 * nsub],
                        mybir.ActivationFunctionType.Exp,
                        scale=SCALE,
                    )
                    for u in range(nsub):
                        j = GS * g + u
                        o = 128 * j - 512 * qb
                        if o > 0:  # zero the fully-masked prefix (stale exp)
                            nc.vector.memset(p_sb[:, 512 * u : 512 * u + o], 0.0)
                        if o >= 0:  # strict-causal mask on the diagonal window
                            nc.vector.tensor_mul(
                                p_sb[:, 512 * u + o : 512 * u + o + 128],
                                p_sb[:, 512 * u + o : 512 * u + o + 128],
                                tm_sb[:, 384:512],
                            )
                    pends.append((g, nsub, p_sb))
                    if len(pends) > 2:
                        first = flush_pv(pends.pop(0), first, last=False)
                    if stage_q:
                        stage_q.pop(0)()
                while pends:
                    first = flush_pv(pends.pop(0), first, last=(not pends))

                st = {"qb": qb, "o_ps": o_ps}
                tail_a(st)
                stage_q.append(lambda st=st: tail_b(st))
                stage_q.append(lambda: None)
                stage_q.append(lambda: None)
                stage_q.append(lambda st=st: tail_c(st))
            while stage_q:
                stage_q.pop(0)()

    nc.finalize()
    return nc


def _get_nc():
    global _CACHED_NC
    if _CACHED_NC is None:
        _CACHED_NC = _build()
    return _CACHED_NC


def _prep_in_maps(inputs):
    f = lambda a: np.ascontiguousarray(np.asarray(a, dtype=np.float32))
    query = f(inputs["query"]).reshape(CQ, N)
    key_feat = f(inputs["key_feat"]).reshape(CK, N)

    def wnorm(v, g):
        v = f(v)
        g = f(g)
        return g[:, None] * v / np.linalg.norm(v, axis=1, keepdims=True)

    wq = wnorm(inputs["vq"], inputs["gq"])
    wk = wnorm(inputs["vk"], inputs["gk"])
    wv = wnorm(inputs["vv"], inputs["gv"])
    bq, bk, bv = f(inputs["bq"]), f(inputs["bk"]), f(inputs["bv"])

    in_maps = []
    for c in range(NH):
        rows = slice(DH * c, DH * (c + 1))
        in_maps.append(
            {
                "qin": query,
                "kin": key_feat,
                "wqt": np.ascontiguousarray(np.tile(wq[rows].T, (1, 4))),
                "wkt": np.ascontiguousarray(np.tile(wk[rows].T, (1, 4))),
                "wvt": np.ascontiguousarray(wv[rows].T),
                "bqr": np.ascontiguousarray(np.tile(bq[rows], 4)[:, None]),
                "bkr": np.ascontiguousarray(np.tile(bk[rows], 4)[:, None]),
                "bvr": np.ascontiguousarray(np.tile(bv[rows][None, :], (128, 1))),
            }
        )
    return in_maps


def _run(inputs, trace=False, **kwargs):
    nc = _get_nc()
    in_maps = _prep_in_maps(inputs)
    res = None
    for attempt in range(3):
        try:
            res = run_bass_kernel_spmd(
                nc, in_maps, core_ids=list(range(NH)), trace=trace, **kwargs
            )
            break
        except Exception:
            if attempt == 2:
                raise

    out = np.empty((B, CH, H, W), dtype=np.float32)
    for c in range(NH):
        oc = res.results[c]["out"]  # [DH, N] (O^T layout)
        out[0, DH * c : DH * (c + 1)] = oc.reshape(DH, H, W)
    return out, res


def kernel(**inputs) -> np.ndarray:
    out, _ = _run(inputs, trace=False)
    return out



# revision 2
# speedup vs baseline: 1.1427x; 1.1427x over previous
"""Trainium2 Bass kernel for nn_CausalAttention (N=4096, 8 heads, DH=32).

Strategy: head-parallel across 8 NeuronCores (1 head per core).
Per core (v2):
  - bf16 inputs/projections (halved DMA + weight loads; PSUM accumulate f32).
  - Scores computed transposed: S^T[k, q] = K @ Q^T, 512-query blocks,
    3 k-tiles (128 keys) per PSUM group (GS=3, row-group packed K=32
    matmuls at partition offsets 0/32/64). Diagonal sub-tiles skip the
    fully-masked column prefix.
  - Max-free softmax: P^T = exp(S / sqrt(32)), strict-causal 0/1 mask
    post-exp (scores O(1): no overflow; -10000 masking underflows to 0).
  - Softmax denominator via ones column appended to V (lhsT [128, 33]).
  - V projection: 4 k-tiles batched into one PSUM bank, single DVE
    bias-add per 512-slice.
  - PV matmuls slice away the causally-dead query prefix on diagonal
    tiles.
  - Input DMAs ride the (otherwise idle) GpSimd queue; tails/weights on
    sync; the Scalar (ACT) queue carries only the 51 exp instructions.
  - Normalization: colsum -> reciprocal -> K=1 matmul broadcast ->
    tensor_mul, staged across following groups to hide DMA latency.
    Output stays O^T [32, 4096] per core; host reshapes.
"""

import math

import numpy as np
import ml_dtypes

import concourse.bass as bass
import concourse.mybir as mybir
from concourse import bacc
from concourse.tile import TileContext
from concourse.bass_utils import run_bass_kernel_spmd

# Problem constants (hardcoded per harness contract).
B, CQ, CK, CH, NH, H, W = 1, 256, 256, 256, 8, 64, 64
DH = CH // NH            # 32
N = H * W                # 4096
QB = 512                 # queries per block
NQB = N // QB            # 8
KT = 128                 # keys per k-tile
NKT = N // KT            # 32
GS = 3                   # k-tiles per S-group (3 PSUM banks per group)
NG = (NKT + GS - 1) // GS  # 11 column-groups in packed kT layout
SCALE = 1.0 / math.sqrt(DH)

F32 = mybir.dt.float32
F32R = mybir.dt.float32r
BF16 = mybir.dt.bfloat16

_CACHED_NC = None


def _build():
    nc = bacc.Bacc("TRN2", target_bir_lowering=False, debug=False, num_devices=1)

    qin_d = nc.dram_tensor("qin", [CQ, N], BF16, kind="ExternalInput")
    kin_d = nc.dram_tensor("kin", [CK, N], BF16, kind="ExternalInput")
    wq_d = nc.dram_tensor("wqt", [CQ, 128], BF16, kind="ExternalInput")
    wk_d = nc.dram_tensor("wkt", [CK, 128], BF16, kind="ExternalInput")
    wv_d = nc.dram_tensor("wvt", [CK, DH], BF16, kind="ExternalInput")
    bq_d = nc.dram_tensor("bqr", [128, 1], F32, kind="ExternalInput")
    bk_d = nc.dram_tensor("bkr", [128, 1], F32, kind="ExternalInput")
    bv_d = nc.dram_tensor("bv4", [128, 128], F32, kind="ExternalInput")
    out_d = nc.dram_tensor("out", [DH, N], F32, kind="ExternalOutput")

    # Strict-causal mask window: tm[kk, j] = 1.0 iff kk < j - 384; the
    # [*, 384:512] slice gives mask[kk, qq] = (kk < qq) for the 128-wide
    # diagonal window.
    tm_np = (np.arange(128)[:, None] < (np.arange(512)[None, :] - 384)).astype(
        ml_dtypes.bfloat16
    )
    tm_d = nc.inline_tensor(tm_np, name="tmask")
    ones_d = nc.inline_tensor(np.ones((1, DH), dtype=np.float32), name="onesd")

    with TileContext(nc) as tc:
        with (
            tc.tile_pool(name="constp", bufs=1) as constp,
            tc.tile_pool(name="bigp", bufs=1) as bigp,
            tc.tile_pool(name="workp", bufs=4) as workp,
            tc.tile_pool(name="spool", bufs=2, space="PSUM") as spool,
            tc.tile_pool(name="ppool", bufs=1, space="PSUM") as ppool,
            tc.tile_pool(name="opool", bufs=1, space="PSUM") as opool,
        ):
            # ---- weights/consts on sync queue (small, needed first) ----
            wk_sb = constp.tile([128, 2, 128], BF16, name="wk_sb")
            nc.sync.dma_start(
                wk_sb[:], wk_d.ap().rearrange("(c p) m -> p c m", p=128)
            )
            wq_sb = constp.tile([128, 2, 128], BF16, name="wq_sb")
            nc.sync.dma_start(
                wq_sb[:], wq_d.ap().rearrange("(c p) m -> p c m", p=128)
            )
            wv_sb = constp.tile([128, 2, DH], BF16, name="wv_sb")
            nc.sync.dma_start(
                wv_sb[:], wv_d.ap().rearrange("(c p) m -> p c m", p=128)
            )
            bq_sb = constp.tile([128, 1], F32, name="bq_sb")
            nc.sync.dma_start(bq_sb[:], bq_d.ap())
            bk_sb = constp.tile([128, 1], F32, name="bk_sb")
            nc.sync.dma_start(bk_sb[:], bk_d.ap())
            bv4_sb = constp.tile([128, 4, DH], F32, name="bv4_sb")
            nc.sync.dma_start(bv4_sb[:], bv_d.ap().rearrange("p (t d) -> p t d", t=4))
            tm_sb = constp.tile([128, 512], BF16, name="tm_sb")
            nc.sync.dma_start(tm_sb[:], tm_d.ap())
            ones_sb = constp.tile([1, DH], F32R, name="ones_sb")
            nc.sync.dma_start(ones_sb[:], ones_d.ap().bitcast(F32R))

            # ---- bulk inputs on the gpsimd (SWDGE) queue, 512-col slices,
            # K first within each slice so K projections never wait ----
            kin_sb = bigp.tile([128, 2, N], BF16, name="kin_sb")
            qin_sb = bigp.tile([128, 2, N], BF16, name="qin_sb")
            kin_ap = kin_d.ap().rearrange("(c p) n -> p c n", p=128)
            qin_ap = qin_d.ap().rearrange("(c p) n -> p c n", p=128)
            for s in range(8):
                sl = slice(512 * s, 512 * (s + 1))
                nc.gpsimd.dma_start(kin_sb[:, :, sl], kin_ap[:, :, sl])
                nc.gpsimd.dma_start(qin_sb[:, :, sl], qin_ap[:, :, sl])

            # Warm the exp activation table before scores exist.
            warm = workp.tile([1, 1], F32, name="warm")
            nc.scalar.activation(
                warm[:], tm_sb[0:1, 0:1], mybir.ActivationFunctionType.Exp
            )

            # kT3[32u+d, 128g+kk] = k^T[d, 128*(3g+u)+kk]  (3-way row packing)
            kT3 = bigp.tile([32 * GS, NG * 128], BF16, name="kT3")
            # qT[32u+d, q] = q^T[d, q] for u=0..3 (4x replicated on partitions)
            qT = bigp.tile([128, N], BF16, name="qT")
            # v_all[kk, t, :DH] = v[128t+kk, :]; col DH is the ones column
            v_all = bigp.tile([128, NKT, 40], BF16, name="v_all")
            nc.vector.memset(v_all[:, :, DH : DH + 1], 1.0)

            stage_q = []  # deferred tail stages, advanced one per group

            def emit_proj(s):
                ksl = slice(512 * s, 512 * (s + 1))
                pj = ppool.tile([128, 512], F32, name="pj", tag="p")
                for ch in range(2):
                    nc.tensor.matmul(
                        pj[:],
                        wk_sb[:, ch, :],
                        kin_sb[:, ch, ksl],
                        start=(ch == 0),
                        stop=(ch == 1),
                    )
                for ci in range(4):
                    j = 4 * s + ci
                    u, g = j % GS, j // GS
                    nc.vector.tensor_scalar_add(
                        kT3[32 * u : 32 * u + 32, 128 * g : 128 * g + 128],
                        pj[32 * u : 32 * u + 32, 128 * ci : 128 * ci + 128],
                        bk_sb[32 * u : 32 * u + 32, :],
                    )
                pj = ppool.tile([128, 512], F32, name="pj", tag="p")
                for ch in range(2):
                    nc.tensor.matmul(
                        pj[:],
                        wq_sb[:, ch, :],
                        qin_sb[:, ch, ksl],
                        start=(ch == 0),
                        stop=(ch == 1),
                    )
                nc.vector.tensor_scalar_add(qT[:, ksl], pj[:], bq_sb[:])
                # V: 4 k-tiles batched into one PSUM bank, one DVE bias-add
                pj = ppool.tile([128, 512], F32, name="pj", tag="p")
                for ti in range(4):
                    t = 4 * s + ti
                    nsl = slice(128 * t, 128 * (t + 1))
                    for ch in range(2):
                        nc.tensor.matmul(
                            pj[:, 32 * ti : 32 * ti + 32],
                            kin_sb[:, ch, nsl],
                            wv_sb[:, ch, :],
                            start=(ch == 0),
                            stop=(ch == 1),
                        )
                nc.vector.tensor_add(
                    v_all[:, 4 * s : 4 * s + 4, 0:DH],
                    pj[:, 0:128].rearrange("p (t d) -> p t d", t=4),
                    bv4_sb[:],
                )

            # ---- block tails, staged into the following group stream so
            # the in-order PE/DVE never stall on the reciprocal DMA chain ----
            def tail_a(st):
                o_ps = st["o_ps"]
                o_sb = workp.tile([DH, 512], F32, name="o_sb")
                nc.vector.tensor_copy(o_sb[:], o_ps[0:DH, :])
                cs_sb = workp.tile([1, 512], F32, name="cs_sb")
                # +1e-30 keeps q=0 (fully masked row) at 0 instead of NaN
                nc.vector.tensor_scalar_add(cs_sb[:], o_ps[DH : DH + 1, :], 1e-30)
                cs8 = workp.tile([8, 64], F32, name="cs8")
                nc.sync.dma_start(cs8[:], cs_sb[:])
                st.update(o_sb=o_sb, cs8=cs8)

            def tail_b(st):
                cs8r = workp.tile([8, 64], F32, name="cs8r")
                nc.vector.reciprocal(cs8r[:], st["cs8"][:])
                csr = workp.tile([1, 512], F32R, name="csr")
                nc.sync.dma_start(csr[:], cs8r[:].bitcast(F32R))
                st.update(csr=csr)

            def tail_c(st):
                qb = st["qb"]
                rep_ps = ppool.tile([DH, 512], F32, name="rep_ps", tag="p")
                nc.tensor.matmul(
                    rep_ps[:], ones_sb[:], st["csr"][:], start=True, stop=True
                )
                out_sb = workp.tile([DH, 512], F32, name="out_sb")
                nc.vector.tensor_mul(out_sb[:], st["o_sb"][:], rep_ps[:])
                nc.sync.dma_start(
                    out_d.ap()[:, 512 * qb : 512 * (qb + 1)], out_sb[:]
                )

            def emit_attn(qb):
                o_ps = opool.tile([DH + 1, 512], F32, name="o_ps", tag="o")
                nkt_q = 4 * (qb + 1)          # causal: k-tiles 0..nkt_q-1
                ngr = (nkt_q + GS - 1) // GS
                pends = []                     # (g, nsub, p_sb) awaiting PV
                first = True

                def flush_pv(pend, first, last, o_ps=o_ps):
                    g, nsub, p_sb = pend
                    f = first
                    for u in range(nsub):
                        j = GS * g + u
                        o = max(0, 128 * j - 512 * qb)
                        if f:
                            o = 0  # first matmul must initialize full PSUM
                        nc.tensor.matmul(
                            o_ps[:, o:512],
                            v_all[:, j, 0 : DH + 1],
                            p_sb[:, 512 * u + o : 512 * (u + 1)],
                            start=f,
                            stop=(last and u == nsub - 1),
                            skip_group_check=True,
                        )
                        f = False
                    return False

                for g in range(ngr):
                    nsub = min(GS, nkt_q - GS * g)
                    s_ps = spool.tile([128, GS * 512], F32, name="s_ps", tag="s")
                    for u in range(nsub):
                        j = GS * g + u
                        o = max(0, 128 * j - 512 * qb)
                        nc.tensor.matmul(
                            s_ps[:, 512 * u + o : 512 * (u + 1)],
                            kT3[32 * u : 32 * u + 32, 128 * g : 128 * g + 128],
                            qT[32 * u : 32 * u + 32, 512 * qb + o : 512 * (qb + 1)],
                            start=True,
                            stop=True,
                        )
                    p_sb = workp.tile([128, GS * 512], BF16, name="p_sb", bufs=8)
                    nc.scalar.activation(
                        p_sb[:, 0 : 512 * nsub],
                        s_ps[:, 0 : 512 * nsub],
                        mybir.ActivationFunctionType.Exp,
                        scale=SCALE,
                    )
                    for u in range(nsub):
                        j = GS * g + u
                        o = 128 * j - 512 * qb
                        if o > 0:  # zero the fully-masked prefix (stale exp)
                            nc.vector.memset(p_sb[:, 512 * u : 512 * u + o], 0.0)
                        if o >= 0:  # strict-causal mask on the diagonal window
                            nc.vector.tensor_mul(
                                p_sb[:, 512 * u + o : 512 * u + o + 128],
                                p_sb[:, 512 * u + o : 512 * u + o + 128],
                                tm_sb[:, 384:512],
                            )
                    pends.append((g, nsub, p_sb))
                    if len(pends) > 2:
                        first = flush_pv(pends.pop(0), first, last=False)
                    if stage_q:
                        stage_q.pop(0)()
                while pends:
                    first = flush_pv(pends.pop(0), first, last=(not pends))

                st = {"qb": qb, "o_ps": o_ps}
                tail_a(st)
                stage_q.append(lambda st=st: tail_b(st))
                stage_q.append(lambda: None)
                stage_q.append(lambda: None)
                stage_q.append(lambda st=st: tail_c(st))

            for s in range(8):
                emit_proj(s)
                emit_attn(s)
            while stage_q:
                stage_q.pop(0)()

    nc.finalize()
    return nc


def _get_nc():
    global _CACHED_NC
    if _CACHED_NC is None:
        _CACHED_NC = _build()
    return _CACHED_NC


def _prep_in_maps(inputs):
    f = lambda a: np.ascontiguousarray(np.asarray(a, dtype=np.float32))
    bf = lambda a: np.ascontiguousarray(a.astype(ml_dtypes.bfloat16))
    query = bf(f(inputs["query"]).reshape(CQ, N))
    key_feat = bf(f(inputs["key_feat"]).reshape(CK, N))

    def wnorm(v, g):
        v = f(v)
        g = f(g)
        return g[:, None] * v / np.linalg.norm(v, axis=1, keepdims=True)

    wq = wnorm(inputs["vq"], inputs["gq"])
    wk = wnorm(inputs["vk"], inputs["gk"])
    wv = wnorm(inputs["vv"], inputs["gv"])
    bq, bk, bv = f(inputs["bq"]), f(inputs["bk"]), f(inputs["bv"])

    in_maps = []
    for c in range(NH):
        rows = slice(DH * c, DH * (c + 1))
        in_maps.append(
            {
                "qin": query,
                "kin": key_feat,
                "wqt": bf(np.tile(wq[rows].T, (1, 4))),
                "wkt": bf(np.tile(wk[rows].T, (1, 4))),
                "wvt": bf(wv[rows].T),
                "bqr": np.ascontiguousarray(np.tile(bq[rows], 4)[:, None]),
                "bkr": np.ascontiguousarray(np.tile(bk[rows], 4)[:, None]),
                "bv4": np.ascontiguousarray(
                    np.tile(bv[rows], (128, 4)).astype(np.float32)
                ),
            }
        )
    return in_maps


def _run(inputs, trace=False, **kwargs):
    nc = _get_nc()
    in_maps = _prep_in_maps(inputs)
    res = None
    for attempt in range(3):
        try:
            res = run_bass_kernel_spmd(
                nc, in_maps, core_ids=list(range(NH)), trace=trace, **kwargs
            )
            break
        except Exception:
            if attempt == 2:
                raise

    out = np.empty((B, CH, H, W), dtype=np.float32)
    for c in range(NH):
        oc = res.results[c]["out"]  # [DH, N] (O^T layout)
        out[0, DH * c : DH * (c + 1)] = oc.reshape(DH, H, W)
    return out, res


def kernel(**inputs) -> np.ndarray:
    out, _ = _run(inputs, trace=False)
    return out
